# revision 13
# baseline (speedup 1.0000x reference)
"""Deformable Conv2d Trainium kernel: host prep + bass program builder.

Sharding: 8 cores = 4 batches x 2 height-halves; each core computes
out[b, :, h0:h0+32, :] (128 out-ch x 8192 positions).

Position layout per core: pos = jj*128 + p, jj in [0,64), p in [0,128);
ho_local = jj//2, wo = (jj%2)*128 + p.

Device pipeline:
  B: offset conv om[27, 8192] (PE, fp16 inputs, fp32 accum)
  C: omT via PE transposes -> OMT [128, 64, 27] f32
  D: bilinear weight math (DVE/ACT) -> V4 [128, 64, 9, 4] fp16, VARF [128,576] f32
  E: idx fold via 8 permutation matmuls -> IDXG [128, 9, 64, 8] int16
  F: per quarter (2048 pos): per tap dma_gather (DRAM table, 512B items)
     -> G [128,16,256] fp16; H = G*V4 (DVE); corner-reduce -> SAMPT;
     PE transposes -> RHS [128,128]; main matmuls -> PSUM -> OUT chunk
"""
import numpy as np

import concourse.bass as bass
import concourse.mybir as mybir
import concourse.tile as tile
from concourse import bacc

F32 = mybir.dt.float32
F16 = mybir.dt.float16
I16 = mybir.dt.int16
I32 = mybir.dt.int32

C = 64
O = 128
H = 64
W = 256
KK = 9
BD = 4                 # vertical halo margin (max |dy| must be < BD)
MX = 5                 # horizontal margin (max |dx| must be < MX)
RT = 42                # table rows: covers y0 in [h0-1-BD, h0+32+BD] inclusive
PITCH = 384            # table row pitch (multiple of 128, >= 270)
TCW = 268              # valid table cols: tc = x0 + MX + 1 in [0, 267]
XCW = TCW + 1          # padded x-slice cols (item needs tc+1)
NIT = RT * PITCH       # table items (16128)
NPOS = 32 * W          # 8192 positions per core
NJJ = 64               # pos blocks of 128
NQ = 4                 # quarters
JQ = 16                # jj per quarter
NF = NJJ * KK          # 576


def _xpad_slice(xb, h0):
    """[C, RT, XCW] zero-padded fp16 slice; rows y_base..y_base+RT-1, col tc=xg+MX+1."""
    y_base = h0 - 1 - BD
    xp = np.zeros((C, RT, XCW), np.float16)
    r0 = max(0, -y_base)
    r1 = min(RT, H - y_base)
    xp[:, r0:r1, MX + 1:MX + 1 + W] = xb[:, y_base + r0:y_base + r1, :].astype(np.float16)
    return xp


def _xp2(xp):
    """[128, RT, XCW]: partitions 0:64 = xp, 64:128 = xp shifted one row up
    (row r holds xp row r+1), so a 128-partition contraction covers vertical
    tap pairs (ki=0,1) in one matmul pass."""
    sh = np.zeros_like(xp)
    sh[:, :RT - 1] = xp[:, 1:]
    return np.concatenate([xp, sh], axis=0)


def _table(xp):
    """[NIT, 256] fp16 gather table; item (r, tc) = per-c [v00, v01, v10, v11]."""
    it = np.zeros((RT, PITCH, C, 4), np.float16)
    it[:RT - 1, :TCW, :, 0] = xp[:, :RT - 1, :TCW].transpose(1, 2, 0)
    it[:RT - 1, :TCW, :, 1] = xp[:, :RT - 1, 1:TCW + 1].transpose(1, 2, 0)
    it[:RT - 1, :TCW, :, 2] = xp[:, 1:RT, :TCW].transpose(1, 2, 0)
    it[:RT - 1, :TCW, :, 3] = xp[:, 1:RT, 1:TCW + 1].transpose(1, 2, 0)
    return it.reshape(NIT, 256)


def host_prep(x, offset_weight, offset_bias, weight):
    """Returns list of 8 in_map dicts; core order = (b, hh)."""
    jj = np.arange(NJJ)
    kv = np.arange(KK)
    ki = kv // 3
    kj = kv % 3
    p = np.arange(128)

    ow2 = offset_weight.reshape(27, C, KK).transpose(1, 2, 0).astype(np.float16)
    # vertical-pair offset-conv weights: row u*64+c of pass kj = w[o, c, ki=u, kj]
    owr = offset_weight.reshape(27, C, 3, 3)
    ow2p2 = np.zeros((128, 3, 27), np.float16)
    for u in range(2):
        for q in range(3):
            ow2p2[u * 64:(u + 1) * 64, q, :] = owr[:, :, u, q].T
    ow2s2 = np.zeros((C, 3, 27), np.float16)
    for q in range(3):
        ow2s2[:, q, :] = owr[:, :, 2, q].T
    ob = offset_bias.reshape(27, 1).astype(np.float32)
    w2 = weight.reshape(O, C, KK)
    w2p = np.zeros((128, 4, 128), np.float16)
    for t in range(4):
        w2p[:64, t, :] = w2[:, :, 2 * t].T.astype(np.float16)
        w2p[64:, t, :] = w2[:, :, 2 * t + 1].T.astype(np.float16)
    w2s = np.ascontiguousarray(w2[:, :, 8].T.astype(np.float16))          # [64, 128]

    x0b = ((jj[None, :, None] % 2) * 128 + p[:, None, None] + kj[None, None, :] - 1
           ).astype(np.float32).reshape(128, NF)
    base = ((jj[None, :, None] // 2 + BD + ki[None, None, :]) * PITCH
            + (jj[None, :, None] % 2) * 128 + p[:, None, None]
            + kj[None, None, :] + MX).astype(np.float32)                   # [128, 64, 9]
    baseg = np.zeros((16, KK, NJJ, 8), np.float32)
    for q in range(8):
        for rr in range(16):
            baseg[rr, :, :, q] = base[q * 16 + rr].T
    baseg = np.tile(baseg, (8, 1, 1, 1)).reshape(128, KK * NJJ * 8)

    permq = np.eye(128, dtype=np.float32)
    idf16 = np.eye(128, dtype=np.float16)
    idf32 = np.eye(27, dtype=np.float32)

    in_maps = []
    for core in range(8):
        b, hh = core // 2, core % 2
        h0 = hh * 32
        xp = _xpad_slice(x[b], h0)
        hoky = ((h0 + jj[None, :, None] // 2 + ki[None, None, :] - 1)
                * np.ones((128, 1, 1))).astype(np.float32)
        in_maps.append({
            "XP2": np.ascontiguousarray(_xp2(xp).reshape(128, RT * XCW)),
            "TBL": np.ascontiguousarray(_table(xp)),
            "OW2P2": np.ascontiguousarray(ow2p2.reshape(128, 3 * 27)),
            "OW2S2": np.ascontiguousarray(ow2s2.reshape(C, 3 * 27)),
            "OB": ob,
            "W2P": w2p, "W2S": w2s,
            "HOKY": np.ascontiguousarray(hoky.reshape(128, NF)), "X0B": x0b,
            "BASEG": baseg, "PERMQ": permq, "IDF16": idf16, "IDF32": idf32,
        })
    return in_maps


def host_post(outs):
    """outs: list of 8 [128, 8192] f32 -> [4, 128, 64, 256]."""
    y = np.zeros((4, O, H, W), np.float32)
    for core, o in enumerate(outs):
        b, hh = core // 2, core % 2
        v = np.asarray(o).reshape(O, 32, 2, 128).reshape(O, 32, 256)
        y[b, :, hh * 32:hh * 32 + 32, :] = v
    return y


def _bcast(ap, dim, n):
    """Insert a [0, n] broadcast dim at free position `dim` (1-based in ap list)."""
    newap = [list(d) for d in ap.ap]
    newap.insert(dim, [0, n])
    return bass.AP(tensor=ap.tensor, offset=ap.offset, ap=newap)


def build(nc=None, upto="full"):
    if nc is None:
        nc = bacc.Bacc("TRN2", target_bir_lowering=False, debug=False)
    if upto in ("idx", "samp", "g1", "h1", "trans", "mm1"):
        DBG = nc.dram_tensor("DBG", [128, KK * NJJ * 8], I16, kind="ExternalOutput")
        SMP = nc.dram_tensor("SMP", [128, JQ * KK * C], F16, kind="ExternalOutput")
    XP2 = nc.dram_tensor("XP2", [128, RT * XCW], F16, kind="ExternalInput")
    TBL = nc.dram_tensor("TBL", [NIT, 256], F16, kind="ExternalInput")
    OW2P2 = nc.dram_tensor("OW2P2", [128, 3 * 27], F16, kind="ExternalInput")
    OW2S2 = nc.dram_tensor("OW2S2", [C, 3 * 27], F16, kind="ExternalInput")
    OB = nc.dram_tensor("OB", [27, 1], F32, kind="ExternalInput")
    W2P = nc.dram_tensor("W2P", [128, 4, 128], F16, kind="ExternalInput")
    W2S = nc.dram_tensor("W2S", [C, 128], F16, kind="ExternalInput")
    HOKY = nc.dram_tensor("HOKY", [128, NF], F32, kind="ExternalInput")
    X0B = nc.dram_tensor("X0B", [128, NF], F32, kind="ExternalInput")
    BASEG = nc.dram_tensor("BASEG", [128, KK * NJJ * 8], F32, kind="ExternalInput")
    PERMQ = nc.dram_tensor("PERMQ", [128, 128], F32, kind="ExternalInput")
    IDF16 = nc.dram_tensor("IDF16", [128, 128], F16, kind="ExternalInput")
    IDF32 = nc.dram_tensor("IDF32", [27, 27], F32, kind="ExternalInput")
    OUT = nc.dram_tensor("OUT", [O, NPOS], F32, kind="ExternalOutput")

    mm = mybir.AluOpType

    with tile.TileContext(nc) as tc:
        with (
            tc.tile_pool(name="const", bufs=1) as cpool,
            tc.tile_pool(name="persist", bufs=1) as ppool,
        ):
            w2p = cpool.tile([128, 4, 128], F16)
            nc.sync.dma_start(out=w2p, in_=W2P[:, :, :])
            w2s = cpool.tile([C, 128], F16)
            nc.sync.dma_start(out=w2s, in_=W2S[:, :])
            idf16 = cpool.tile([128, 128], F16)
            nc.sync.dma_start(out=idf16, in_=IDF16[:, :])

            omt = ppool.tile([128, NJJ, 27], F32)
            v4 = ppool.tile([128, NJJ, KK, 4], F16)
            idxg = ppool.tile([128, KK, NJJ, 8], I16)

            # ================= phases B-E =================
            with (
                tc.tile_pool(name="wk", bufs=1) as wk,
                tc.tile_pool(name="psA", bufs=2, space="PSUM") as psA,
            ):
                xp2 = wk.tile([128, RT, XCW], F16)
                nc.sync.dma_start(out=xp2,
                                  in_=XP2[:, :].rearrange("c (r w) -> c r w", r=RT))
                ow2p2 = wk.tile([128, 3, 27], F16)
                nc.sync.dma_start(out=ow2p2,
                                  in_=OW2P2[:, :].rearrange("c (k o) -> c k o", k=3))
                ow2s2 = wk.tile([C, 3, 27], F16)
                nc.sync.dma_start(out=ow2s2,
                                  in_=OW2S2[:, :].rearrange("c (k o) -> c k o", k=3))
                ob = wk.tile([27, 1], F32)
                nc.sync.dma_start(out=ob, in_=OB[:, :])
                hoky = wk.tile([128, NF], F32)
                nc.sync.dma_start(out=hoky, in_=HOKY[:, :])
                x0b = wk.tile([128, NF], F32)
                nc.sync.dma_start(out=x0b, in_=X0B[:, :])
                baseg = wk.tile([128, KK * NJJ * 8], F32)
                nc.sync.dma_start(out=baseg, in_=BASEG[:, :])
                permq = wk.tile([128, 128], F32)
                nc.sync.dma_start(out=permq, in_=PERMQ[:, :])
                idf32 = wk.tile([27, 27], F32)
                nc.sync.dma_start(out=idf32, in_=IDF32[:, :])

                # ---- B: offset conv (vertical tap pairs via row-shifted
                # partition copy: 6 passes instead of 9) ----
                om_s = wk.tile([27, NPOS], F32)
                for ch in range(16):
                    ps = psA.tile([27, 512], F32, tag="psom")
                    r = 2 * ch + BD
                    for kj in range(3):
                        rhs = xp2[:, r:r + 2, kj + MX:kj + MX + W]
                        nc.tensor.matmul(ps, ow2p2[:, kj, :], rhs,
                                         start=(kj == 0), stop=False)
                    for kj in range(3):
                        rhs = xp2[0:C, r + 2:r + 4, kj + MX:kj + MX + W]
                        nc.tensor.matmul(ps, ow2s2[:, kj, :], rhs,
                                         start=False, stop=(kj == 2))
                    nc.vector.tensor_scalar(out=om_s[:, ch * 512:(ch + 1) * 512],
                                            in0=ps, scalar1=ob, scalar2=None,
                                            op0=mm.add)

                # ---- C: transpose om -> OMT [128, 64, 27] ----
                for jj in range(NJJ):
                    pst = psA.tile([128, 27], F32, tag="psomt")
                    nc.tensor.transpose(pst, om_s[:, jj * 128:(jj + 1) * 128], idf32)
                    nc.any.tensor_copy(omt[:, jj, :], pst)

                # ---- D: weight math ----
                DY = omt[:, :, 0:18:2]
                DX = omt[:, :, 1:18:2]
                MZ = omt[:, :, 18:27]

                msig = wk.tile([128, NF], F32)
                nc.scalar.activation(out=msig, in_=MZ,
                                     func=mybir.ActivationFunctionType.Sigmoid)

                def floor_frac(src_ap, ftag):
                    # robust floor under any f32->i32 rounding mode:
                    # t = cast(src); f = t - (src < t); l = src - f
                    ti = wk.tile([128, NF], I32, tag="flr_i")
                    nc.vector.tensor_copy(ti, src_ap)
                    tf = wk.tile([128, NF], F32, tag="flr_f")
                    nc.vector.tensor_copy(tf, ti)
                    lt = wk.tile([128, NF], F32, tag="flr_lt")
                    nc.vector.tensor_tensor(out=lt, in0=src_ap, in1=tf,
                                            op=mm.is_lt)
                    fl = wk.tile([128, NF], F32, tag=ftag + "_f")
                    nc.vector.tensor_tensor(out=fl, in0=tf, in1=lt,
                                            op=mm.subtract)
                    fr = wk.tile([128, NF], F32, tag=ftag + "_l")
                    nc.vector.tensor_tensor(out=fr, in0=src_ap, in1=fl,
                                            op=mm.subtract)
                    return fl, fr

                fy, ly = floor_frac(DY, "fy")
                fx, lx = floor_frac(DX, "fx")

                y0 = wk.tile([128, NF], F32)
                nc.vector.tensor_tensor(out=y0, in0=fy, in1=hoky, op=mm.add)
                x0 = wk.tile([128, NF], F32)
                nc.vector.tensor_tensor(out=x0, in0=fx, in1=x0b, op=mm.add)

                def in_range(src, lo, hi, out_tag):
                    a = wk.tile([128, NF], F32, tag="rng_a")
                    nc.vector.tensor_scalar(out=a, in0=src, scalar1=float(lo),
                                            scalar2=None, op0=mm.is_ge)
                    bq = wk.tile([128, NF], F32, tag="rng_b")
                    nc.vector.tensor_scalar(out=bq, in0=src, scalar1=float(hi),
                                            scalar2=None, op0=mm.is_le)
                    r = wk.tile([128, NF], F32, tag=out_tag)
                    nc.vector.tensor_tensor(out=r, in0=a, in1=bq, op=mm.mult)
                    return r

                vy0 = in_range(y0, 0, H - 1, "vy0")
                vy1 = in_range(y0, -1, H - 2, "vy1")
                vx0 = in_range(x0, 0, W - 1, "vx0")
                vx1 = in_range(x0, -1, W - 2, "vx1")

                ily = wk.tile([128, NF], F32)
                nc.vector.tensor_scalar(out=ily, in0=ly, scalar1=-1.0, scalar2=1.0,
                                        op0=mm.mult, op1=mm.add)
                ilx = wk.tile([128, NF], F32)
                nc.vector.tensor_scalar(out=ilx, in0=lx, scalar1=-1.0, scalar2=1.0,
                                        op0=mm.mult, op1=mm.add)

                a0 = wk.tile([128, NF], F32)
                nc.vector.tensor_tensor(out=a0, in0=msig, in1=ily, op=mm.mult)
                nc.vector.tensor_tensor(out=a0, in0=a0, in1=vy0, op=mm.mult)
                a1 = wk.tile([128, NF], F32)
                nc.vector.tensor_tensor(out=a1, in0=msig, in1=ly, op=mm.mult)
                nc.vector.tensor_tensor(out=a1, in0=a1, in1=vy1, op=mm.mult)
                b0 = wk.tile([128, NF], F32)
                nc.vector.tensor_tensor(out=b0, in0=ilx, in1=vx0, op=mm.mult)
                b1 = wk.tile([128, NF], F32)
                nc.vector.tensor_tensor(out=b1, in0=lx, in1=vx1, op=mm.mult)

                nc.vector.tensor_tensor(out=v4[:, :, :, 0], in0=a0, in1=b0, op=mm.mult)
                nc.vector.tensor_tensor(out=v4[:, :, :, 1], in0=a0, in1=b1, op=mm.mult)
                nc.vector.tensor_tensor(out=v4[:, :, :, 2], in0=a1, in1=b0, op=mm.mult)
                nc.vector.tensor_tensor(out=v4[:, :, :, 3], in0=a1, in1=b1, op=mm.mult)

                varf = wk.tile([128, NF], F32)
                nc.vector.scalar_tensor_tensor(out=varf, in0=fy, scalar=float(PITCH),
                                               in1=fx, op0=mm.mult, op1=mm.add)

                # ---- E: idx fold ----
                varg = wk.tile([16, KK, NJJ, 8], F32)
                for q in range(8):
                    for hh in range(2):
                        psf = psA.tile([16, 288], F32, tag="psfold")
                        nc.tensor.matmul(psf, permq[:, q * 16:(q + 1) * 16],
                                         varf[:, hh * 288:(hh + 1) * 288],
                                         start=True, stop=True)
                        # psf free = (jj in [hh*32..), k); dst (rr, k, jj, q)
                        src = bass.AP(tensor=psf.tensor, offset=psf[:, 0].offset,
                                      ap=[list(psf[:, :].ap[0]), [1, KK], [KK, 32]])
                        dst = bass.AP(tensor=varg.tensor,
                                      offset=varg[0, 0, hh * 32, q].offset,
                                      ap=[[varg[:, 0, 0, 0].ap[0][0], 16],
                                          [NJJ * 8, KK], [8, 32]])
                        nc.any.tensor_copy(dst, src)

                nc.vector.tensor_tensor(out=idxg[0:16, :, :, :],
                                        in0=varg[:, :, :, :],
                                        in1=baseg[0:16, :].rearrange(
                                            "r (k j q) -> r k j q", k=KK, j=NJJ),
                                        op=mm.add)
                for lo, n in ((16, 16), (32, 32), (64, 64)):
                    nc.sync.dma_start(out=idxg[lo:lo + n, :, :, :],
                                      in_=idxg[0:n, :, :, :])
            if upto == "idx":
                nc.sync.dma_start(out=DBG[:, :], in_=idxg[:, :, :, :])

            # ================= phase F =================
            if upto == "idx":
                pass
            elif upto in ("g1", "h1"):
                with (
                    tc.tile_pool(name="gat", bufs=3) as gpool,
                    tc.tile_pool(name="hb", bufs=2) as hpool,
                ):
                    k, Q = 0, 0
                    g = gpool.tile([128, JQ, 256], F16)
                    nc.gpsimd.dma_gather(
                        out_ap=g[:, :, :], in_ap=TBL[:, :],
                        idxs_ap=idxg[:, k, Q * JQ:(Q + 1) * JQ, :],
                        num_idxs=JQ * 128, num_idxs_reg=JQ * 128,
                        elem_size=256, single_packet=False)
                    if upto == "h1":
                        v4s = v4[:, Q * JQ:(Q + 1) * JQ, k, :]
                        v4v = _bcast(v4s, 2, C)
                        h = hpool.tile([128, JQ, C, 4], F16)
                        nc.vector.tensor_tensor(
                            out=h,
                            in0=g[:, :, :].rearrange("p j (c f) -> p j c f", c=C),
                            in1=v4v, op=mm.mult)
                        nc.sync.dma_start(out=SMP[:, 0:JQ * C * 4], in_=h)
                    else:
                        nc.sync.dma_start(out=SMP[:, 0:JQ * 256], in_=g)
            elif upto == "samp":
                with (
                    tc.tile_pool(name="gat", bufs=3) as gpool,
                    tc.tile_pool(name="hb", bufs=2) as hpool,
                    tc.tile_pool(name="samp", bufs=2) as spool,
                ):
                    sampt = spool.tile([128, JQ, KK, C], F16)
                    Q = 0
                    for k in range(KK):
                        g = gpool.tile([128, JQ, 256], F16)
                        nc.gpsimd.dma_gather(
                            out_ap=g[:, :, :], in_ap=TBL[:, :],
                            idxs_ap=idxg[:, k, Q * JQ:(Q + 1) * JQ, :],
                            num_idxs=JQ * 128, num_idxs_reg=JQ * 128,
                            elem_size=256, single_packet=False)
                        v4s = v4[:, Q * JQ:(Q + 1) * JQ, k, :]
                        v4v = _bcast(v4s, 2, C)
                        h = hpool.tile([128, JQ, C, 4], F16)
                        nc.vector.tensor_tensor(
                            out=h,
                            in0=g[:, :, :].rearrange("p j (c f) -> p j c f", c=C),
                            in1=v4v, op=mm.mult)
                        with nc.allow_low_precision(reason="4-corner fp16 sum"):
                            nc.vector.tensor_reduce(
                                out=sampt[:, :, k, :],
                                in_=h[:, :, :, :].rearrange("p j c f -> p (j c) f"),
                                axis=mybir.AxisListType.X, op=mm.add)
                    nc.sync.dma_start(out=SMP[:, :], in_=sampt)
            elif upto == "trans":
                with (
                    tc.tile_pool(name="gat", bufs=3) as gpool,
                    tc.tile_pool(name="hb", bufs=2) as hpool,
                    tc.tile_pool(name="samp", bufs=2) as spool,
                    tc.tile_pool(name="rhs", bufs=6) as rpool,
                    tc.tile_pool(name="psT", bufs=3, space="PSUM") as psT,
                ):
                    sampt = spool.tile([128, JQ, KK, C], F16)
                    Q = 0
                    for k in range(KK):
                        g = gpool.tile([128, JQ, 256], F16)
                        nc.gpsimd.dma_gather(
                            out_ap=g[:, :, :], in_ap=TBL[:, :],
                            idxs_ap=idxg[:, k, Q * JQ:(Q + 1) * JQ, :],
                            num_idxs=JQ * 128, num_idxs_reg=JQ * 128,
                            elem_size=256, single_packet=False)
                        v4s = v4[:, Q * JQ:(Q + 1) * JQ, k, :]
                        v4v = _bcast(v4s, 2, C)
                        h = hpool.tile([128, JQ, C, 4], F16)
                        nc.vector.tensor_tensor(
                            out=h,
                            in0=g[:, :, :].rearrange("p j (c f) -> p j c f", c=C),
                            in1=v4v, op=mm.mult)
                        with nc.allow_low_precision(reason="4-corner fp16 sum"):
                            nc.vector.tensor_reduce(
                                out=sampt[:, :, k, :],
                                in_=h[:, :, :, :].rearrange("p j c f -> p (j c) f"),
                                axis=mybir.AxisListType.X, op=mm.add)
                    for jl in range(2):
                        for t in range(4):
                            pst2 = psT.tile([128, 128], F16, tag="pstp")
                            nc.tensor.transpose(
                                pst2, sampt[:, jl, 2 * t:2 * t + 2, :], idf16)
                            rhs_t = rpool.tile([128, 128], F16, tag="rhs")
                            nc.any.tensor_copy(rhs_t, pst2)
                            nc.sync.dma_start(
                                out=SMP[:, (jl * 4 + t) * 128:(jl * 4 + t + 1) * 128],
                                in_=rhs_t)
            elif upto == "mm1":
                with (
                    tc.tile_pool(name="gat", bufs=3) as gpool,
                    tc.tile_pool(name="hb", bufs=2) as hpool,
                    tc.tile_pool(name="samp", bufs=2) as spool,
                    tc.tile_pool(name="rhs", bufs=6) as rpool,
                    tc.tile_pool(name="psT", bufs=3, space="PSUM") as psT,
                    tc.tile_pool(name="psO", bufs=2, space="PSUM") as psO,
                ):
                    sampt = spool.tile([128, JQ, KK, C], F16)
                    Q = 0
                    for k in range(KK):
                        g = gpool.tile([128, JQ, 256], F16)
                        nc.gpsimd.dma_gather(
                            out_ap=g[:, :, :], in_ap=TBL[:, :],
                            idxs_ap=idxg[:, k, Q * JQ:(Q + 1) * JQ, :],
                            num_idxs=JQ * 128, num_idxs_reg=JQ * 128,
                            elem_size=256, single_packet=False)
                        v4s = v4[:, Q * JQ:(Q + 1) * JQ, k, :]
                        v4v = _bcast(v4s, 2, C)
                        h = hpool.tile([128, JQ, C, 4], F16)
                        nc.vector.tensor_tensor(
                            out=h,
                            in0=g[:, :, :].rearrange("p j (c f) -> p j c f", c=C),
                            in1=v4v, op=mm.mult)
                        with nc.allow_low_precision(reason="4-corner fp16 sum"):
                            nc.vector.tensor_reduce(
                                out=sampt[:, :, k, :],
                                in_=h[:, :, :, :].rearrange("p j c f -> p (j c) f"),
                                axis=mybir.AxisListType.X, op=mm.add)
                    for jl in range(2):
                        pso = psO.tile([O, 128], F32, tag="pso")
                        rhs_list = []
                        for t in range(4):
                            pst2 = psT.tile([128, 128], F16, tag="pstp")
                            nc.tensor.transpose(
                                pst2, sampt[:, jl, 2 * t:2 * t + 2, :], idf16)
                            rhs_t = rpool.tile([128, 128], F16, tag="rhs")
                            nc.any.tensor_copy(rhs_t, pst2)
                            rhs_list.append(rhs_t)
                        pst3 = psT.tile([C, 128], F16, tag="psts")
                        nc.tensor.transpose(pst3, sampt[:, jl, 8, :], idf16)
                        rhs_s = rpool.tile([C, 128], F16, tag="rhss")
                        nc.any.tensor_copy(rhs_s, pst3)
                        for t in range(4):
                            nc.tensor.matmul(pso, w2p[:, t, :], rhs_list[t],
                                             start=(t == 0), stop=False)
                        nc.tensor.matmul(pso, w2s, rhs_s, start=False, stop=True)
                        oqm = rpool.tile([O, 128], F32, tag="oqm")
                        nc.any.tensor_copy(oqm, pso)
                        nc.sync.dma_start(out=OUT[:, jl * 128:(jl + 1) * 128],
                                          in_=oqm)
            elif upto == "full":
              with (
                  tc.tile_pool(name="gat", bufs=4) as gpool,
                  tc.tile_pool(name="hb", bufs=2) as hpool,
                  tc.tile_pool(name="samp", bufs=2) as spool,
                  tc.tile_pool(name="rhs", bufs=12) as rpool,
                  tc.tile_pool(name="oq", bufs=2) as opool,
                  tc.tile_pool(name="psT", bufs=3, space="PSUM") as psT,
                  tc.tile_pool(name="psO", bufs=2, space="PSUM") as psO,
              ):
                  def jl_loop(sampt, Q):
                      oq = opool.tile([O, JQ, 128], F32)
                      for jl in range(JQ):
                          rhs_list = []
                          for t in range(4):
                              pst2 = psT.tile([128, 128], F16, tag="pstp")
                              nc.tensor.transpose(
                                  pst2, sampt[:, jl, 2 * t:2 * t + 2, :], idf16)
                              rhs_t = rpool.tile([128, 128], F16, tag="rhs")
                              nc.scalar.copy(rhs_t, pst2)
                              rhs_list.append(rhs_t)
                          pst3 = psT.tile([C, 128], F16, tag="psts")
                          nc.tensor.transpose(pst3, sampt[:, jl, 8, :], idf16)
                          rhs_s = rpool.tile([C, 128], F16, tag="rhss")
                          nc.scalar.copy(rhs_s, pst3)
                          pso = psO.tile([O, 128], F32, tag="pso")
                          with tc.tile_critical():
                              for t in range(4):
                                  nc.tensor.matmul(pso, w2p[:, t, :], rhs_list[t],
                                                   start=(t == 0), stop=False)
                              nc.tensor.matmul(pso, w2s, rhs_s,
                                               start=False, stop=True)
                          nc.any.tensor_copy(oq[:, jl, :], pso)
                      nc.sync.dma_start(
                          out=OUT[:, Q * 2048:(Q + 1) * 2048].rearrange(
                              "o (j p) -> o j p", j=JQ),
                          in_=oq)

                  # Software-pipelined: quarter Q's gathers + DVE sampling run
                  # (in program order) BEFORE quarter Q-1's matmul tail, so the
                  # strict-FIFO DVE queue never parks next-Q work behind the
                  # PE transpose/matmul chain.
                  prev = None
                  prevQ = -1
                  for Q in range(NQ):
                      sampt = spool.tile([128, JQ, KK, C], F16)
                      for k in range(KK):
                          g = gpool.tile([128, JQ, 256], F16)
                          nc.gpsimd.dma_gather(
                              out_ap=g[:, :, :],
                              in_ap=TBL[:, :],
                              idxs_ap=idxg[:, k, Q * JQ:(Q + 1) * JQ, :],
                              num_idxs=JQ * 128,
                              num_idxs_reg=JQ * 128,
                              elem_size=256, single_packet=False,
                          )
                          v4s = v4[:, Q * JQ:(Q + 1) * JQ, k, :]
                          v4v = _bcast(v4s, 2, C)
                          h = hpool.tile([128, JQ, C, 4], F16, tag="h")
                          nc.vector.tensor_tensor(
                              out=h, in0=g[:, :, :].rearrange("p j (c f) -> p j c f", c=C),
                              in1=v4v, op=mm.mult)
                          # 4-corner sum as two adds: first runs in DVE 2x mode
                          # (packed fp16 pairs), beats tensor_reduce's 1x.
                          s2 = hpool.tile([128, JQ, C, 2], F16, tag="s2")
                          with nc.allow_low_precision(reason="4-corner fp16 sum"):
                              nc.vector.tensor_tensor(
                                  out=s2, in0=h[:, :, :, 0:2],
                                  in1=h[:, :, :, 2:4], op=mm.add)
                              nc.vector.tensor_tensor(
                                  out=sampt[:, :, k, :], in0=s2[:, :, :, 0],
                                  in1=s2[:, :, :, 1], op=mm.add)
                          if k == 2 and prev is not None:
                              # previous quarter's matmul tail, issued early so
                              # it overlaps this quarter's remaining gathers
                              jl_loop(prev, prevQ)
                              prev = None
                      prev, prevQ = sampt, Q
                  jl_loop(prev, prevQ)
    nc.compile()
    return nc


# ======================= runner =======================
_NC = None


def _get_nc():
    global _NC
    if _NC is None:
        _NC = build()
    return _NC


def kernel(x, offset_weight, offset_bias, weight):
    from concourse.bass_utils import run_bass_kernel_spmd
    x = np.asarray(x)
    offset_weight = np.asarray(offset_weight)
    offset_bias = np.asarray(offset_bias)
    weight = np.asarray(weight)
    in_maps = host_prep(x, offset_weight, offset_bias, weight)
    nc = _get_nc()
    res = run_bass_kernel_spmd(nc, in_maps, core_ids=list(range(8)))
    return host_post([r["OUT"] for r in res.results])

